# revision 13
# baseline (speedup 1.0000x reference)
"""CartBonded whole-pose scoring on 8 Trainium2 NeuronCores — v4.

Sharding (pose-major per hint): core c owns poses [8c, 8c+8).
Host: buckets terms by pose, gathers per-term atom coords, forms the
per-term edge vectors (bond delta / angle u,v / torsion b1,b2,b3 —
scaled by 1/16 so all intermediates fit fp16) plus per-term params
(bond: sqrt(K), x0/16; angle: sqrt(K), x0-pi/2; torsion: K, cos(x0),
-sin(x0)), and packs each (pose-PAIR, type) into fp16 plane-major
blocks [P, nplanes, 2F] (geometry and params separate) so every
device instruction covers two poses.
Device: fp16 DVE (2x mode) with a 3-stage software pipeline across
pose pairs so cross-engine round-trips (ACT rsqrt/arctan, Pool
cross-product mults) hide behind the next pair's independent DVE
work.  Per-pose energies accumulate into f32 partials columns via ACT
Square+accum (bond/angle) and the AFFINE_MUL_REDUCE custom DVE op
(torsion); final cross-partition reduce is a ones-vector matmul on PE
(bond column rescaled by 256 to undo the coordinate scaling).
"""

import numpy as np

N_POSES = 64
MAX_ATOMS = 16384
N_CORES = 8
PP = N_POSES // N_CORES
NPAIR = PP // 2
P = 128
PI = float(np.pi)
SCL = 1.0 / 16.0

_BUILD_CACHE = {}


# ----------------------------------------------------------------- host prep
def _bucketize(atoms):
    n = atoms.shape[0]
    pose = (atoms[:, 0] // MAX_ATOMS).astype(np.int64)
    order = np.argsort(pose, kind="stable")
    pose_s = pose[order]
    counts = np.bincount(pose, minlength=N_POSES)
    F = -(-int(counts.max()) // P)
    F = -(-F // 4) * 4
    starts = np.zeros(N_POSES + 1, np.int64)
    np.cumsum(counts, out=starts[1:])
    r = np.arange(n, dtype=np.int64) - starts[pose_s]
    part = (r // F).astype(np.int64)
    free = (r % F).astype(np.int64)
    assert part.max() < P
    return order, pose_s, part, free, F


def _pack(pose_s, part, free, F, vals16):
    """[N_POSES,P,npl,F] -> pair-major [N_POSES//2, P, npl, 2F]."""
    npl = vals16.shape[1]
    X = np.zeros((N_POSES, P, npl, F), np.float16)
    X[pose_s, part, :, free] = vals16
    X = X.reshape(N_POSES // 2, 2, P, npl, F)
    X = np.ascontiguousarray(X.transpose(0, 2, 3, 1, 4))
    return X.reshape(N_POSES // 2, P, npl, 2 * F)


# --------------------------------------------------------------- device build
def _build(Fb, Fa, Ft):
    key = (Fb, Fa, Ft)
    if key in _BUILD_CACHE:
        return _BUILD_CACHE[key]

    import concourse.bass as bass
    import concourse.tile as tile
    from concourse import bacc, mybir
    from concourse.dve_ops import AFFINE_MUL_REDUCE

    dt = mybir.dt
    f16 = dt.float16
    f32 = dt.float32
    Act = mybir.ActivationFunctionType
    Op = mybir.AluOpType

    Gb, Ga, Gt = 2 * Fb, 2 * Fa, 2 * Ft

    nc = bacc.Bacc("TRN2", target_bir_lowering=False, debug=False,
                   num_devices=N_CORES)

    bg = nc.dram_tensor("bg", [NPAIR, P, 3, Gb], f16,
                        kind="ExternalInput").ap()
    bp = nc.dram_tensor("bp", [NPAIR, P, 2, Gb], f16,
                        kind="ExternalInput").ap()
    ag = nc.dram_tensor("ag", [NPAIR, P, 6, Ga], f16,
                        kind="ExternalInput").ap()
    apq = nc.dram_tensor("apq", [NPAIR, P, 2, Ga], f16,
                         kind="ExternalInput").ap()
    tg = nc.dram_tensor("tg", [NPAIR, P, 9, Gt], f16,
                        kind="ExternalInput").ap()
    tpp = nc.dram_tensor("tpp", [NPAIR, P, 3, Gt], f16,
                         kind="ExternalInput").ap()
    out = nc.dram_tensor("out", [1, PP], f32, kind="ExternalOutput").ap()

    cz = nc.alloc_sbuf_tensor("constf32-zero", [P, 1], f32)
    nc.gpsimd.memset(cz.ap(), 0.0)
    nc.const_aps.aps[(f32, 0.0)] = cz.ap()
    nc.all_engine_barrier()

    from contextlib import ExitStack

    with tile.TileContext(nc) as tc, ExitStack() as ctx:
        pers = ctx.enter_context(tc.tile_pool(name="pers", bufs=1))
        inp = ctx.enter_context(tc.tile_pool(name="inp", bufs=2))
        par = ctx.enter_context(tc.tile_pool(name="par", bufs=3))
        tp = ctx.enter_context(tc.tile_pool(name="tmp", bufs=1))
        xp = ctx.enter_context(tc.tile_pool(name="xe", bufs=2))
        psum = ctx.enter_context(tc.tile_pool(name="ps", bufs=1, space="PSUM"))

        partials = pers.tile([P, PP * 3], f32)

        V = nc.vector
        PL = nc.gpsimd
        SC = nc.scalar

        def T2(tag, shape):       # DVE-only temps (bufs=1)
            return tp.tile(shape, f16, tag=tag, name=tag)

        def X2(tag, shape):       # cross-engine temps (bufs=2)
            return xp.tile(shape, f16, tag=tag, name=tag)

        def X4(tag, shape):
            return xp.tile(shape, f32, tag=tag, name=tag)

        def rsqrt(out_ap, in_ap):
            E = SC
            ins = [E.lower_ap(in_ap),
                   E.lower_ap(nc.const_aps.scalar_like(0.0, in_ap)),
                   mybir.ImmediateValue(dtype=f32, value=1.0),
                   mybir.ImmediateValue(dtype=f32, value=0.0)]
            return E.add_instruction(mybir.InstActivation(
                name=nc.get_next_instruction_name(), func=Act.Rsqrt,
                ins=ins, outs=[E.lower_ap(out_ap)]))

        ctxs = [dict() for _ in range(NPAIR)]

        def load(q):
            c = ctxs[q]
            btg = inp.tile([P, 3, Gb], f16, tag="btg", name="btg")
            nc.sync.dma_start(btg[:], bg[q])
            atg = inp.tile([P, 6, Ga], f16, tag="atg", name="atg")
            nc.sync.dma_start(atg[:], ag[q])
            ttg = inp.tile([P, 9, Gt], f16, tag="ttg", name="ttg")
            nc.sync.dma_start(ttg[:], tg[q])
            btp = par.tile([P, 2, Gb], f16, tag="btp", name="btp")
            nc.sync.dma_start(btp[:], bp[q])
            atp = par.tile([P, 2, Ga], f16, tag="atp", name="atp")
            nc.sync.dma_start(atp[:], apq[q])
            ttp = par.tile([P, 3, Gt], f16, tag="ttp", name="ttp")
            nc.sync.dma_start(ttp[:], tpp[q])
            c.update(btg=btg, atg=atg, ttg=ttg, btp=btp, atp=atp, ttp=ttp)
            b2 = ttg[:, 3:6, :]
            b3 = ttg[:, 6:9, :]
            n2q = X2("n2q", [P, 3, Gt])
            n2w = X2("n2w", [P, 3, Gt])
            for k in range(3):
                PL.tensor_tensor(out=n2q[:, k, :], in0=b2[:, (k + 1) % 3, :],
                                 in1=b3[:, (k + 2) % 3, :], op=Op.mult)
            for k in range(3):
                PL.tensor_tensor(out=n2w[:, k, :], in0=b2[:, (k + 2) % 3, :],
                                 in1=b3[:, (k + 1) % 3, :], op=Op.mult)
            c["n2q"], c["n2w"] = n2q, n2w
            # ACT head start on the input squares
            X6 = X2("X6", [P, 6, Gt])    # [tm | sq(b2)]
            SC.activation(X6[:, 3:6, :], b2, Act.Square)
            c["X6"] = X6
            sqb = X2("sqb", [P, 3, Gb])
            SC.activation(sqb[:], btg[:], Act.Square)
            c["sqb"] = sqb
            squv = X2("squv", [P, 6, Ga])
            SC.activation(squv[:], atg[:], Act.Square)
            c["squv"] = squv

        def emitA(q):
            c = ctxs[q]
            ttg, atg = c["ttg"], c["atg"]
            b1 = ttg[:, 0:3, :]
            b2 = ttg[:, 3:6, :]
            b3 = ttg[:, 6:9, :]
            n2 = T2("n2", [P, 3, Gt])
            V.tensor_tensor(out=n2[:], in0=c["n2q"][:], in1=c["n2w"][:],
                            op=Op.subtract)
            n1q = T2("n1q", [P, 3, Gt])
            n1w = T2("n1w", [P, 3, Gt])
            n1 = T2("n1", [P, 3, Gt])
            for k in range(3):
                V.tensor_tensor(out=n1q[:, k, :], in0=b1[:, (k + 1) % 3, :],
                                in1=b2[:, (k + 2) % 3, :], op=Op.mult)
            for k in range(3):
                V.tensor_tensor(out=n1w[:, k, :], in0=b1[:, (k + 2) % 3, :],
                                in1=b2[:, (k + 1) % 3, :], op=Op.mult)
            V.tensor_tensor(out=n1[:], in0=n1q[:], in1=n1w[:], op=Op.subtract)
            X6 = c["X6"]
            V.tensor_tensor(out=X6[:, 0:3, :], in0=n1[:], in1=b3, op=Op.mult)
            bm = tp.tile([P, 3, Gt], f16, tag="n1q", name="bm")
            V.tensor_tensor(out=bm[:], in0=n1[:], in1=n2[:], op=Op.mult)
            AB = xp.tile([P, 2, Gt], f16, tag="AB", name="AB", bufs=3)
            Bt = T2("Bt", [P, Gt])
            V.tensor_tensor(out=Bt[:], in0=bm[:, 0, :], in1=bm[:, 1, :],
                            op=Op.add)
            V.tensor_tensor(out=AB[:, 1, :], in0=Bt[:], in1=bm[:, 2, :],
                            op=Op.add)
            TS2 = T2("TS2", [P, 2, Gt])
            V.tensor_tensor(out=TS2[:], in0=X6[:, 0::3, :],
                            in1=X6[:, 1::3, :], op=Op.add)
            V.tensor_tensor(out=TS2[:], in0=TS2[:], in1=X6[:, 2::3, :],
                            op=Op.add)
            S2c = X2("S2c", [P, Gt])
            V.tensor_scalar(out=S2c[:], in0=TS2[:, 1, :], scalar1=1e-4,
                            scalar2=None, op0=Op.add)
            rs2 = X2("rs2", [P, Gt])
            rsqrt(rs2[:], S2c[:])
            t1 = T2("t1", [P, Gt])
            V.tensor_tensor(out=t1[:], in0=TS2[:, 0, :], in1=S2c[:],
                            op=Op.mult)
            c["rs2"], c["t1"], c["AB"] = rs2, t1, AB
            # bond D2
            sqb = c["sqb"]
            D2 = X2("D2", [P, Gb])
            V.tensor_tensor(out=D2[:], in0=sqb[:, 0, :], in1=sqb[:, 1, :],
                            op=Op.add)
            V.tensor_tensor(out=D2[:], in0=D2[:], in1=sqb[:, 2, :], op=Op.add)
            V.tensor_scalar(out=D2[:], in0=D2[:], scalar1=1e-4, scalar2=None,
                            op0=Op.add)
            rsb = X2("rsb", [P, Gb])
            rsqrt(rsb[:], D2[:])
            c["D2"], c["rsb"] = D2, rsb
            # angle head
            squv = c["squv"]
            u2v2 = T2("u2v2", [P, 2, Ga])
            V.tensor_tensor(out=u2v2[:], in0=squv[:, 0::3, :],
                            in1=squv[:, 1::3, :], op=Op.add)
            V.tensor_tensor(out=u2v2[:], in0=u2v2[:], in1=squv[:, 2::3, :],
                            op=Op.add)
            xm = tp.tile([P, 3, Ga], f16, tag="TS2", name="xm")
            V.tensor_tensor(out=xm[:], in0=atg[:, 0:3, :], in1=atg[:, 3:6, :],
                            op=Op.mult)
            xa = X2("xa", [P, Ga])
            V.tensor_tensor(out=xa[:], in0=xm[:, 0, :], in1=xm[:, 1, :],
                            op=Op.add)
            V.tensor_tensor(out=xa[:], in0=xa[:], in1=xm[:, 2, :], op=Op.add)
            x2 = T2("x2", [P, Ga])
            V.tensor_tensor(out=x2[:], in0=xa[:], in1=xa[:], op=Op.mult)
            Sa = X2("Sa", [P, Ga])
            V.tensor_tensor(out=Sa[:], in0=u2v2[:, 0, :], in1=u2v2[:, 1, :],
                            op=Op.mult)
            V.tensor_tensor(out=Sa[:], in0=Sa[:], in1=x2[:], op=Op.subtract)
            # S can dip negative in fp16 (u2*v2 - x2 cancellation)
            V.tensor_scalar(out=Sa[:], in0=Sa[:], scalar1=1e-4, scalar2=None,
                            op0=Op.max)
            iy = X2("iy", [P, Ga])
            rsqrt(iy[:], Sa[:])
            c["xa"], c["iy"] = xa, iy

        def emitB(q):
            c = ctxs[q]
            p0 = q * 2
            AB = c["AB"]
            V.tensor_tensor(out=AB[:, 0, :], in0=c["t1"][:], in1=c["rs2"][:],
                            op=Op.mult)
            AB2 = X4("AB2", [P, 2, Gt])
            SC.activation(AB2[:], AB[:], Act.Square)
            c["AB2"] = AB2
            # bond tail
            btp = c["btp"]
            dd = X2("ddb", [P, Gb])
            V.tensor_tensor(out=dd[:], in0=c["D2"][:], in1=c["rsb"][:],
                            op=Op.mult)
            V.tensor_tensor(out=dd[:], in0=dd[:], in1=btp[:, 1, :],
                            op=Op.subtract)
            V.tensor_tensor(out=dd[:], in0=dd[:], in1=btp[:, 0, :],
                            op=Op.mult)
            scrb = T2("scrb", [P, Gb])
            for k in (0, 1):
                SC.activation(scrb[:, k * Fb:(k + 1) * Fb],
                              dd[:, k * Fb:(k + 1) * Fb], Act.Square,
                              accum_out=partials[:, (p0 + k) * 3:
                                                 (p0 + k) * 3 + 1])
            ra = X2("ra", [P, Ga])
            V.tensor_tensor(out=ra[:], in0=c["xa"][:], in1=c["iy"][:],
                            op=Op.mult)
            th = X2("th", [P, Ga])
            SC.activation(th[:], ra[:], Act.Arctan)
            c["th"] = th

        def emitC(q):
            c = ctxs[q]
            p0 = q * 2
            AB2 = c["AB2"]
            R2c = X4("R2c", [P, Gt])
            V.scalar_tensor_tensor(out=R2c[:], in0=AB2[:, 0, :], scalar=1e-8,
                                   in1=AB2[:, 1, :], op0=Op.add, op1=Op.add)
            iR = X2("iR", [P, Gt])
            rsqrt(iR[:], R2c[:])
            # fill DVE while ACT computes iR: angle tail
            atp, th = c["atp"], c["th"]
            tha = T2("tha", [P, Ga])
            V.tensor_tensor(out=tha[:], in0=th[:], in1=atp[:, 1, :],
                            op=Op.add)
            V.tensor_tensor(out=tha[:], in0=tha[:], in1=atp[:, 0, :],
                            op=Op.mult)
            scra = X2("scra", [P, Ga])
            for k in (0, 1):
                SC.activation(scra[:, k * Fa:(k + 1) * Fa],
                              tha[:, k * Fa:(k + 1) * Fa], Act.Square,
                              accum_out=partials[:, (p0 + k) * 3 + 1:
                                                 (p0 + k) * 3 + 2])
            AB, ttp = c["AB"], c["ttp"]
            cs = tp.tile([P, 2, Gt], f16, tag="n1", name="cs")
            V.tensor_tensor(out=cs[:, 0, :], in0=AB[:, 1, :], in1=iR[:],
                            op=Op.mult)
            V.tensor_tensor(out=cs[:, 1, :], in0=AB[:, 0, :], in1=iR[:],
                            op=Op.mult)
            cs2 = tp.tile([P, 2, Gt], f16, tag="n1w", name="cs2")
            V.tensor_tensor(out=cs2[:], in0=cs[:], in1=cs[:], op=Op.mult)
            tqt = tp.tile([P, 2, Gt], f16, tag="n2", name="tqt")
            V.tensor_scalar(out=tqt[:, 0, :], in0=cs2[:, 0, :], scalar1=4.0,
                            scalar2=-3.0, op0=Op.mult, op1=Op.add)
            V.tensor_scalar(out=tqt[:, 1, :], in0=cs2[:, 1, :], scalar1=-4.0,
                            scalar2=3.0, op0=Op.mult, op1=Op.add)
            c3s3 = tp.tile([P, 2, Gt], f16, tag="u2v2", name="c3s3")
            V.tensor_tensor(out=c3s3[:], in0=cs[:], in1=tqt[:], op=Op.mult)
            wv = tp.tile([P, 2, Gt], f16, tag="x2", name="wv")
            V.tensor_tensor(out=wv[:], in0=c3s3[:], in1=ttp[:, 1:3, :],
                            op=Op.mult)
            ut = T2("ut", [P, Gt])
            V.tensor_tensor(out=ut[:], in0=wv[:, 0, :], in1=wv[:, 1, :],
                            op=Op.add)
            scrt = T2("scrt", [P, Gt])
            for k in (0, 1):
                V._custom_dve(AFFINE_MUL_REDUCE,
                              out=scrt[:, k * Ft:(k + 1) * Ft],
                              in0=ut[:, k * Ft:(k + 1) * Ft],
                              in1=ttp[:, 0, k * Ft:(k + 1) * Ft],
                              s0=1.0, s1=1.0,
                              accum_out=partials[:, (p0 + k) * 3 + 2:
                                                 (p0 + k) * 3 + 3])

        load(0)
        for s in range(NPAIR + 2):
            if 0 <= s - 1 < NPAIR:
                emitB(s - 1)
            if 0 <= s - 2 < NPAIR:
                emitC(s - 2)
            if s < NPAIR:
                emitA(s)
            if s + 1 < NPAIR:
                load(s + 1)

        ones = pers.tile([P, 1], f32)
        V.memset(ones[:], 1.0)
        ps = psum.tile([1, PP * 3], f32)
        nc.tensor.matmul(out=ps[:], lhsT=ones[:], rhs=partials[:],
                         start=True, stop=True)
        psc = pers.tile([1, PP * 3], f32)
        V.tensor_copy(out=psc[:], in_=ps[:])
        s8 = pers.tile([1, PP], f32)
        # bond column carries K*(d/16 - x0/16)^2 -> scale by 256
        V.tensor_scalar(out=s8[:], in0=psc[0:1, 0:PP * 3:3], scalar1=256.0,
                        scalar2=None, op0=Op.mult)
        V.tensor_tensor(out=s8[:], in0=s8[:], in1=psc[0:1, 1:PP * 3:3],
                        op=Op.add)
        V.tensor_tensor(out=s8[:], in0=s8[:], in1=psc[0:1, 2:PP * 3:3],
                        op=Op.add)
        nc.sync.dma_start(out[:], s8[:])

    nc.compile()
    _BUILD_CACHE[key] = nc
    return nc


# ---------------------------------------------------------------------- main
def kernel(coords, global_params, bond_x0, angle_x0, tor_x0,
           bond_atoms, bond_param_idx, angle_atoms, angle_param_idx,
           tor_atoms, tor_param_idx, _trace=False):
    coords = np.asarray(coords, dtype=np.float32)
    K_table = np.asarray(global_params, dtype=np.float32)[:, 0]
    flat = (coords.reshape(-1, 3) * SCL).astype(np.float32)

    # ---- bond ----
    atoms = np.asarray(bond_atoms)
    order, pose_s, part, free, Fb = _bucketize(atoms)
    g = flat[atoms[order].astype(np.int64)]            # [n,2,3]
    gv = (g[:, 0] - g[:, 1]).astype(np.float16)
    pv = np.empty((len(order), 2), np.float32)
    pv[:, 0] = np.sqrt(K_table[np.asarray(bond_param_idx)[order]])
    pv[:, 1] = np.asarray(bond_x0, np.float32)[order] * SCL
    Xbg = _pack(pose_s, part, free, Fb, gv)
    Xbp = _pack(pose_s, part, free, Fb, pv.astype(np.float16))

    # ---- angle ----
    atoms = np.asarray(angle_atoms)
    order, pose_s, part, free, Fa = _bucketize(atoms)
    g = flat[atoms[order].astype(np.int64)]            # [n,3,3]
    gv = np.empty((len(order), 6), np.float32)
    gv[:, 0:3] = g[:, 0] - g[:, 1]
    gv[:, 3:6] = g[:, 2] - g[:, 1]
    pv = np.empty((len(order), 2), np.float32)
    pv[:, 0] = np.sqrt(K_table[np.asarray(angle_param_idx)[order]])
    pv[:, 1] = np.asarray(angle_x0, np.float32)[order] - PI / 2
    Xag = _pack(pose_s, part, free, Fa, gv.astype(np.float16))
    Xap = _pack(pose_s, part, free, Fa, pv.astype(np.float16))

    # ---- torsion ----
    atoms = np.asarray(tor_atoms)
    order, pose_s, part, free, Ft = _bucketize(atoms)
    g = flat[atoms[order].astype(np.int64)]            # [n,4,3]
    gv = np.empty((len(order), 9), np.float32)
    gv[:, 0:3] = g[:, 1] - g[:, 0]
    gv[:, 3:6] = g[:, 2] - g[:, 1]
    gv[:, 6:9] = g[:, 3] - g[:, 2]
    pv = np.empty((len(order), 3), np.float32)
    pv[:, 0] = K_table[np.asarray(tor_param_idx)[order]]
    x0s = np.asarray(tor_x0, np.float32)[order]
    pv[:, 1] = np.cos(x0s)
    pv[:, 2] = -np.sin(x0s)  # device's s = -sin(phi); s3 odd in s
    Xtg = _pack(pose_s, part, free, Ft, gv.astype(np.float16))
    Xtp = _pack(pose_s, part, free, Ft, pv.astype(np.float16))

    nc = _build(Fb, Fa, Ft)

    in_maps = []
    for c in range(N_CORES):
        lo, hi = c * NPAIR, (c + 1) * NPAIR
        in_maps.append({
            "bg": np.ascontiguousarray(Xbg[lo:hi]),
            "bp": np.ascontiguousarray(Xbp[lo:hi]),
            "ag": np.ascontiguousarray(Xag[lo:hi]),
            "apq": np.ascontiguousarray(Xap[lo:hi]),
            "tg": np.ascontiguousarray(Xtg[lo:hi]),
            "tpp": np.ascontiguousarray(Xtp[lo:hi]),
        })

    from concourse.bass_utils import run_bass_kernel_spmd
    res = run_bass_kernel_spmd(nc, in_maps, list(range(N_CORES)),
                               trace=_trace)
    out = np.concatenate([res.results[c]["out"][0] for c in range(N_CORES)])
    if _trace:
        kernel._last_result = res
    return out.astype(np.float32)
